# revision 15
# baseline (speedup 1.0000x reference)
# Distributed kNN-retrieval loss kernel for Trainium2 (8 NeuronCores).
#
# Reference: two linear heads + softmax, feature bank updated at trg_idx
# (no-grad), cosine kNN against the bank, KL pseudo-label + entropy/IM +
# label-smoothed CE. Output: scalar loss.
#
# v3 strategy:
#  * Bank shipped fp8(e4m3) [D, 25088]/core (25000 rows + 88 zero-pad cols);
#    stream matmuls are DoubleRow fp8 (contraction 256/instr): half the PE
#    instructions and HBM bytes of bf16.
#  * Per-row positive scaling never changes a row's top-k order, so the
#    stream uses UNNORMALIZED trg_feat.
#  * Index-in-value encoding kills FIND_INDEX8: ACT (and DVE for the odd
#    tile) copy each PSUM dist tile as fp16 into the HIGH u16 halves of an
#    f32 canvas whose LOW halves hold the window-local column id. For the
#    positive values that matter, f32 order of the packed word ==
#    (fp16 dist, col) lexicographic: ONE MAX8 per 3584-col window returns
#    values AND indices, tie-free.
#  * KL-through-the-collective: each core decodes its local top-8, gathers
#    their score rows from its own (patched) score_bank and reduces them to
#    per-candidate KL contributions WHILE the AllGather rendezvous waits;
#    the single 16KB AllGather carries (enc2, kl) pairs where enc2 =
#    (fp16 val | unique global slot id). Post-collective work is just a
#    MAX8 + 5 exact-match selects - no index exchange, no post-merge
#    gathers.
#  * Bank update handled via G = fn@fn.T (exact, f32): all updated columns
#    compete as candidates with slots 64..71; their KL comes from
#    KLG[b,j] = H_a[j] - (Pa Pa^T)[b,j] selected by the same match trick
#    (zero gathers). Self column = global max, dropped: exactly
#    reference's top_k(K+1)[:,1:]. Stale stream columns stay (~2e-4).
#  * Entropy/IM/CE computed in the stream shadow; host reads core 0's loss.

import ml_dtypes
import numpy as np

import concourse.bass as bass
import concourse.mybir as mybir
import concourse.tile as tile
from concourse import bacc
from concourse.bass import IndirectOffsetOnAxis
from concourse.bass_utils import run_bass_kernel_spmd

F32 = mybir.dt.float32
F16 = mybir.dt.float16
BF16 = mybir.dt.bfloat16
FP8 = mybir.dt.float8e4
U32 = mybir.dt.uint32
U16 = mybir.dt.uint16
AF = mybir.ActivationFunctionType
ALU = mybir.AluOpType
AX = mybir.AxisListType
DR = mybir.MatmulPerfMode.DoubleRow

B = 256
D = 512
C = 10
N = 200000
K = 5
EPS_LS = 0.1
ENT_WT, IM_WT, AAD_WT, TGT_WT = 1.0, 1.0, 1.0, 0.1

P = 128
NM = B // P
NCORES = 8
NLOC = N // NCORES        # 25000
TN = 512
NTW = 7
WWIN = TN * NTW           # 3584
NWIN = 7
NLOCP = NWIN * WWIN       # 25088
TOT = NCORES * 8 + 8      # 72

BIGNEG = -1.0e30


def hi16(ap):
    """[P, n] f32 AP -> [P, n] u16 AP of the high halves."""
    return ap.bitcast(U16).rearrange("p (j two) -> p j two", two=2)[:, :, 1]


def lo16(ap):
    return ap.bitcast(U16).rearrange("p (j two) -> p j two", two=2)[:, :, 0]


def build_program(debug=False):
    nc = bacc.Bacc(
        "TRN2", target_bir_lowering=False, debug=False, num_devices=NCORES
    )

    fb8_h = nc.dram_tensor("fb8", [D, NLOCP], FP8, kind="ExternalInput")
    tf8_h = nc.dram_tensor("tf8", [D, B], FP8, kind="ExternalInput")
    tfT_h = nc.dram_tensor("tfT", [D, B], F32, kind="ExternalInput")
    cb_h = nc.dram_tensor("core_base", [P, 1], F32, kind="ExternalInput")
    gmask_h = nc.dram_tensor("gmask", [1, B], F32, kind="ExternalInput")
    Wm_h = nc.dram_tensor("Wm", [D, C], F32, kind="ExternalInput")
    bm_h = nc.dram_tensor("bm", [1, C], F32, kind="ExternalInput")
    Wa_h = nc.dram_tensor("Wa", [D, C], F32, kind="ExternalInput")
    ba_h = nc.dram_tensor("ba", [1, C], F32, kind="ExternalInput")
    sb_h = nc.dram_tensor("sbank", [N, C], F32, kind="ExternalInput")
    ramp_h = nc.dram_tensor("ramp", [1, WWIN], U32, kind="ExternalInput")
    slotlo_h = nc.dram_tensor("slotlo", [1, 16], U32, kind="ExternalInput")
    winoff_h = nc.dram_tensor("winoff", [1, NWIN * 8], F32, kind="ExternalInput")
    eye_h = nc.dram_tensor("eye", [P, P], F32, kind="ExternalInput")
    tidxu_h = nc.dram_tensor("tidxu", [B, 1], U32, kind="ExternalInput")
    loss_h = nc.dram_tensor("loss", [1, 1], F32, kind="ExternalOutput")

    def dump(name, ap):
        if not debug:
            return
        t = nc.dram_tensor(f"dbg_{name}", list(ap.shape), ap.dtype,
                           kind="ExternalOutput")
        nc.sync.dma_start(t.ap()[tuple(slice(0, d) for d in ap.shape)], ap)

    with tile.TileContext(nc) as tc:
        with (
            tc.tile_pool(name="const", bufs=1) as cp,
            tc.tile_pool(name="fbw", bufs=3) as fp,
            tc.tile_pool(name="scratch", bufs=2) as sp,
            tc.tile_pool(name="psA", bufs=2, space="PSUM") as ppA,
            tc.tile_pool(name="psS", bufs=2, space="PSUM") as ppS,
            tc.tile_pool(name="psB", bufs=1, space="PSUM") as ppB,
            tc.tile_pool(name="dram", bufs=1, space="DRAM") as dr,
        ):
            # ---------- gating transfers, fewest possible descriptors ----------
            # (Sync descgen is ~600ns per dma_start and fully serial, so the
            # stream-gating loads are consolidated and emitted first.)
            tf8_t = cp.tile([P, 4, B], FP8, tag="tf8", name="tf8")
            nc.sync.dma_start(
                tf8_t[:], tf8_h.ap().rearrange("(s p) b -> p s b", s=4),
            )
            tf8 = [tf8_t[:, 2 * kp:2 * kp + 2, :] for kp in range(2)]
            ramp_sb = cp.tile([1, WWIN], U32, tag="ramp", name="ramp")
            nc.sync.dma_start(ramp_sb[:], ramp_h.ap()[:, :])
            tfT_t = cp.tile([P, 4, B], F32, tag="tfT", name="tfT")
            nc.sync.dma_start(
                tfT_t[:], tfT_h.ap().rearrange("(s p) b -> p s b", s=4),
            )
            tfT = [tfT_t[:, k, :] for k in range(4)]
            Wsb = {}
            bsb = {}
            for name, Wh, bh in (("m", Wm_h, bm_h), ("a", Wa_h, ba_h)):
                Wt = cp.tile([P, 4, C], F32, tag=f"W{name}", name=f"W{name}")
                nc.sync.dma_start(
                    Wt[:], Wh.ap().rearrange("(s p) c -> p s c", s=4),
                )
                Wsb[name] = [Wt[:, k, :] for k in range(4)]
                bsb[name] = cp.tile([1, C], F32, tag=f"b{name}", name=f"b{name}")
                nc.sync.dma_start(bsb[name][:], bh.ap()[:, :])

            # window 0/1 bank data
            fbw_all = []
            HW2 = WWIN // 2
            for w in range(2):
                fbw = [fp.tile([P, 2, WWIN], FP8, tag=f"fbw{kp}",
                               name=f"fbw{kp}_{w}") for kp in range(2)]
                for h in range(2):      # halves: first tiles arrive sooner
                    for kp in range(2):
                        nc.sync.dma_start(
                            fbw[kp][:, :, h * HW2:(h + 1) * HW2],
                            fb8_h.ap()[kp * 256:(kp + 1) * 256,
                                       w * WWIN + h * HW2:
                                       w * WWIN + (h + 1) * HW2].rearrange(
                                "(s p) j -> p s j", s=2),
                        )
                fbw_all.append(fbw)

            # canvases' low halves (GpSimd, overlaps the DMAs above)
            ebuf = [[cp.tile([P, WWIN], F32, tag=f"ebuf{m}{bi}",
                             name=f"ebuf{m}{bi}") for bi in range(2)]
                    for m in range(NM)]
            for bi in range(2):          # buffer 0 first: window 0 needs it
                for m in range(NM):
                    nc.gpsimd.partition_broadcast(
                        ebuf[m][bi][:].bitcast(U32), ramp_sb[:]
                    )

            # late-use constants (after the stream-gating DMAs)
            gmask_sb = cp.tile([1, B], F32, tag="gmask", name="gmask")
            nc.sync.dma_start(gmask_sb[:], gmask_h.ap()[:, :])
            cb_sb = cp.tile([P, 1], F32, tag="cb", name="cb")
            nc.sync.dma_start(cb_sb[:], cb_h.ap()[:, :])
            tidxu_sb = [cp.tile([P, 1], U32, tag=f"tidxu{m}", name=f"tidxu{m}")
                        for m in range(NM)]
            for m in range(NM):
                nc.sync.dma_start(tidxu_sb[m][:], tidxu_h.ap()[m * P:(m + 1) * P, :])
            slotlo_sb = cp.tile([1, 16], U32, tag="slotlo", name="slotlo")
            nc.sync.dma_start(slotlo_sb[:], slotlo_h.ap()[:, :])
            winoff_sb = cp.tile([1, NWIN * 8], F32, tag="winoff", name="winoff")
            nc.sync.dma_start(winoff_sb[:], winoff_h.ap()[:, :])
            winoff_bc = cp.tile([P, NWIN * 8], F32, tag="winoff_bc",
                                name="winoff_bc")
            nc.gpsimd.partition_broadcast(winoff_bc[:], winoff_sb[:])
            eye_sb = cp.tile([P, P], F32, tag="eye", name="eye")
            nc.sync.dma_start(eye_sb[:], eye_h.ap()[:, :])

            ones_k1 = cp.tile([1, P], F32, tag="ones_k1", name="ones_k1")
            nc.vector.memset(ones_k1[:], 1.0)
            ones_m1 = cp.tile([P, 1], F32, tag="ones_m1", name="ones_m1")
            nc.vector.memset(ones_m1[:], 1.0)
            eps_b = cp.tile([P, 1], F32, tag="eps_b", name="eps_b")
            nc.vector.memset(eps_b[:], 1e-5)

            # ---------- classifier heads + softmax ----------
            p_t = {"m": [], "a": []}
            pmax_t = {"m": [], "a": []}
            logp_t = []
            for m in range(NM):
                msl = slice(m * P, (m + 1) * P)
                for name in ("m", "a"):
                    ps = ppB.tile([P, C], F32, tag="ps_small", name="ps_small")
                    for k in range(4):
                        nc.tensor.matmul(
                            ps[:], lhsT=tfT[k][:, msl], rhs=Wsb[name][k],
                            start=(k == 0), stop=False,
                        )
                    nc.tensor.matmul(
                        ps[:], lhsT=ones_k1[:], rhs=bsb[name][:],
                        start=False, stop=True,
                    )
                    lg = cp.tile([P, C], F32, tag=f"lg{name}{m}", name=f"lg{name}{m}")
                    nc.scalar.copy(lg[:], ps[:])
                    mx = cp.tile([P, 1], F32, tag=f"mx{name}{m}", name=f"mx{name}{m}")
                    nc.vector.reduce_max(mx[:], lg[:], axis=AX.X)
                    negmx = sp.tile([P, 1], F32, tag="negmx", name="negmx")
                    nc.vector.tensor_scalar_mul(negmx[:], mx[:], -1.0)
                    exps = sp.tile([P, C], F32, tag="exps", name="exps")
                    sumexp = cp.tile([P, 1], F32, tag=f"se{name}{m}",
                                     name=f"se{name}{m}")
                    nc.scalar.activation(
                        exps[:], lg[:], AF.Exp, bias=negmx[:], scale=1.0,
                        accum_out=sumexp[:],
                    )
                    rcp = sp.tile([P, 1], F32, tag="rcp", name="rcp")
                    nc.vector.reciprocal(rcp[:], sumexp[:])
                    pp = cp.tile([P, C], F32, tag=f"p{name}{m}", name=f"p{name}{m}")
                    nc.vector.tensor_scalar_mul(pp[:], exps[:], rcp[:])
                    p_t[name].append(pp)
                    pm = cp.tile([P, 1], F32, tag=f"pmax{name}{m}",
                                 name=f"pmax{name}{m}")
                    nc.vector.reduce_max(pm[:], pp[:], axis=AX.X)
                    pmax_t[name].append(pm)
                    if name == "m":
                        lnS = sp.tile([P, 1], F32, tag="lnS", name="lnS")
                        nc.scalar.activation(lnS[:], sumexp[:], AF.Ln)
                        logZ = sp.tile([P, 1], F32, tag="logZ", name="logZ")
                        nc.vector.tensor_add(logZ[:], lnS[:], mx[:])
                        lp = cp.tile([P, C], F32, tag=f"logp{m}", name=f"logp{m}")
                        nc.vector.tensor_scalar_sub(lp[:], lg[:], logZ[:])
                        logp_t.append(lp)

            # patch this core's score_bank: rows trg_idx <- p_aad
            for m in range(NM):
                nc.gpsimd.indirect_dma_start(
                    out=sb_h.ap(),
                    out_offset=IndirectOffsetOnAxis(ap=tidxu_sb[m][:], axis=0),
                    in_=p_t["a"][m][:],
                    in_offset=None,
                )

            # ---------- G = trg @ trg.T scaled to raw-dist column scale ----------
            ps2 = ppB.tile([1, B], F32, tag="ps_small", name="ps_s2")
            for k in range(4):
                sq = sp.tile([P, B], F32, tag="sq", name="sq")
                nc.scalar.square(sq[:], tfT[k])
                nc.tensor.matmul(
                    ps2[:], lhsT=ones_m1[:], rhs=sq[:],
                    start=(k == 0), stop=(k == 3),
                )
            srow = cp.tile([1, B], F32, tag="srow", name="srow")
            nc.scalar.sqrt(srow[:], ps2[:])
            invs = cp.tile([1, B], F32, tag="invs", name="invs")
            nc.vector.reciprocal(invs[:], srow[:])
            invs_bc = cp.tile([P, B], F32, tag="invs_bc", name="invs_bc")
            nc.gpsimd.partition_broadcast(invs_bc[:], invs[:])
            gm_bc = cp.tile([P, B], F32, tag="gm_bc", name="gm_bc")
            nc.gpsimd.partition_broadcast(gm_bc[:], gmask_sb[:])

            # Pa^T [C, B] via PE transpose (for Pa@Pa^T and H_a)
            paT = cp.tile([C, B], F32, tag="paT", name="paT")
            for m in range(NM):
                pst = ppB.tile([C, P], F32, tag="ps_small", name="ps_tr")
                nc.tensor.transpose(pst[:], p_t["a"][m][:], eye_sb[:])
                nc.scalar.copy(paT[:, m * P:(m + 1) * P], pst[:])
            # H_a[j] = sum_c pa[j,c] ln pa[j,c]  as a [1, B] row
            lnpaT = sp.tile([C, B], F32, tag="lnpaT", name="lnpaT")
            nc.scalar.activation(lnpaT[:], paT[:], AF.Ln)
            pelnT = sp.tile([C, B], F32, tag="pelnT", name="pelnT")
            nc.vector.tensor_mul(pelnT[:], paT[:], lnpaT[:])
            psH = ppB.tile([1, B], F32, tag="ps_small", name="ps_H")
            nc.tensor.matmul(psH[:], lhsT=ones_m1[0:C, :], rhs=pelnT[:],
                             start=True, stop=True)
            Ha = cp.tile([1, B], F32, tag="Ha", name="Ha")
            nc.scalar.copy(Ha[:], psH[:])
            Ha_bc = cp.tile([P, B], F32, tag="Ha_bc", name="Ha_bc")
            nc.gpsimd.partition_broadcast(Ha_bc[:], Ha[:])

            Gtop = []
            klG = []
            for m in range(NM):
                msl = slice(m * P, (m + 1) * P)
                psG = ppB.tile([P, B], F32, tag="ps_small", name="ps_G")
                for k in range(4):
                    nc.tensor.matmul(
                        psG[:], lhsT=tfT[k][:, msl], rhs=tfT[k],
                        start=(k == 0), stop=(k == 3),
                    )
                Gp = sp.tile([P, B], F32, tag="Gp", name="Gp")
                nc.vector.tensor_mul(Gp[:], psG[:], invs_bc[:])
                nc.vector.tensor_add(Gp[:], Gp[:], gm_bc[:])
                gt = cp.tile([P, 8], F32, tag=f"Gtop{m}", name=f"Gtop{m}")
                nc.vector.max(out=gt[:], in_=Gp[:])
                # KLG[b, j] = H_a[j] - (Pa Pa^T)[b, j]
                psPP = ppB.tile([P, B], F32, tag="ps_small", name="ps_PP")
                nc.tensor.matmul(psPP[:], lhsT=paT[:, msl], rhs=paT[:],
                                 start=True, stop=True)
                KLG = sp.tile([P, B], F32, tag="KLG", name="KLG")
                nc.vector.tensor_sub(KLG[:], Ha_bc[:], psPP[:])
                kg = cp.tile([P, 8], F32, tag=f"klG{m}", name=f"klG{m}")
                for gs in range(8):
                    gtmp = sp.tile([P, B], F32, tag="gtmp", name="gtmp")
                    gred = sp.tile([P, 1], F32, tag="gred", name="gred")
                    nc.vector.scalar_tensor_tensor(
                        gtmp[:], in0=Gp[:], scalar=gt[:, gs:gs + 1],
                        in1=KLG[:], op0=ALU.is_equal, op1=ALU.mult,
                        accum_out=gred[:],
                    )
                    nc.vector.tensor_copy(kg[:, gs:gs + 1], gred[:])
                Gtop.append(gt)
                klG.append(kg)
                dump(f"gtop{m}", gt[:])
                dump(f"klG{m}", kg[:])

            # ---------- entropy / IM / CE (stream shadow) ----------
            stat = [cp.tile([P, 3], F32, tag=f"stat{m}", name=f"stat{m}")
                    for m in range(NM)]
            for m in range(NM):
                lp5 = sp.tile([P, C], F32, tag="lp5", name="lp5")
                nc.scalar.activation(lp5[:], p_t["m"][m][:], AF.Ln, bias=eps_b[:])
                pe = sp.tile([P, C], F32, tag="pe", name="pe")
                nc.vector.tensor_mul(pe[:], p_t["m"][m][:], lp5[:])
                entneg = sp.tile([P, 1], F32, tag="entneg", name="entneg")
                nc.vector.reduce_sum(entneg[:], pe[:], axis=AX.X)
                nc.vector.tensor_copy(stat[m][:, 1:2], entneg[:])

                pickm = sp.tile([P, 1], U32, tag="pickm", name="pickm")
                nc.vector.tensor_tensor(
                    pickm[:], pmax_t["m"][m][:], pmax_t["a"][m][:], op=ALU.is_gt
                )
                chosen = sp.tile([P, C], F32, tag="chosen", name="chosen")
                nc.vector.select(
                    chosen[:], pickm[:].to_broadcast([P, C]),
                    p_t["m"][m][:], p_t["a"][m][:],
                )
                c8 = sp.tile([P, 8], F32, tag="c8", name="c8")
                nc.vector.max(out=c8[:], in_=chosen[:])
                ohlp = sp.tile([P, C], F32, tag="ohlp", name="ohlp")
                lpsel = sp.tile([P, 1], F32, tag="lpsel", name="lpsel")
                nc.vector.scalar_tensor_tensor(
                    ohlp[:], in0=chosen[:], scalar=c8[:, 0:1],
                    in1=logp_t[m][:], op0=ALU.is_equal, op1=ALU.mult,
                    accum_out=lpsel[:],
                )
                slogp = sp.tile([P, 1], F32, tag="slogp", name="slogp")
                nc.vector.reduce_sum(slogp[:], logp_t[m][:], axis=AX.X)
                sl001 = sp.tile([P, 1], F32, tag="sl001", name="sl001")
                nc.vector.tensor_scalar_mul(sl001[:], slogp[:], EPS_LS / C)
                cerow = sp.tile([P, 1], F32, tag="cerow", name="cerow")
                nc.vector.scalar_tensor_tensor(
                    cerow[:], in0=lpsel[:], scalar=(1.0 - EPS_LS), in1=sl001[:],
                    op0=ALU.mult, op1=ALU.add,
                )
                nc.vector.tensor_scalar_mul(cerow[:], cerow[:], -1.0)
                nc.vector.tensor_copy(stat[m][:, 2:3], cerow[:])

            ps_mp = ppB.tile([1, C], F32, tag="ps_small", name="ps_mp")
            for m in range(NM):
                nc.tensor.matmul(
                    ps_mp[:], lhsT=ones_m1[:], rhs=p_t["m"][m][:],
                    start=(m == 0), stop=(m == NM - 1),
                )
            mp = cp.tile([1, C], F32, tag="mp", name="mp")
            nc.scalar.mul(mp[:], ps_mp[:], 1.0 / B)
            mplog = sp.tile([1, C], F32, tag="mplog", name="mplog")
            nc.scalar.activation(mplog[:], mp[:], AF.Ln, bias=eps_b[0:1, :])
            mpe = sp.tile([1, C], F32, tag="mpe", name="mpe")
            nc.vector.tensor_mul(mpe[:], mp[:], mplog[:])
            imsum = cp.tile([1, 1], F32, tag="imsum", name="imsum")
            nc.vector.reduce_sum(imsum[:], mpe[:], axis=AX.X)

            # ---------- local merge + per-candidate KL (pre-collective) ----------
            e2 = []
            klc = []
            def local_merge(m):
                lv8 = sp.tile([P, 8], F32, tag="lv8", name="lv8", bufs=2)
                nc.vector.max(out=lv8[:], in_=canv[m][:])
                woff = sp.tile([P, 8], F32, tag="woff", name="woff", bufs=2)
                for s in range(5):
                    tmp = sp.tile([P, NWIN * 8], F32, tag="tmpw", name="tmpw",
                                  bufs=3)
                    red = sp.tile([P, 1], F32, tag="redw", name="redw", bufs=3)
                    nc.vector.scalar_tensor_tensor(
                        tmp[:], in0=canv[m][:], scalar=lv8[:, s:s + 1],
                        in1=winoff_bc[:], op0=ALU.is_equal, op1=ALU.mult,
                        accum_out=red[:],
                    )
                    nc.vector.tensor_copy(woff[:, s:s + 1], red[:])
                jlocf = sp.tile([P, 5], F32, tag="jlocf", name="jlocf", bufs=2)
                nc.vector.tensor_copy(jlocf[:], lo16(lv8[:])[:, 0:5])
                idx = sp.tile([P, 5], F32, tag="idx", name="idx", bufs=2)
                nc.vector.tensor_add(idx[:], jlocf[:], woff[:, 0:5])
                nc.vector.tensor_scalar_add(idx[:], idx[:], cb_sb[:])
                nc.vector.tensor_scalar_min(idx[:], idx[:], float(N - 1))
                bidx_u = sp.tile([P, 5], U32, tag="bidxu", name="bidxu", bufs=2)
                nc.vector.tensor_copy(bidx_u[:], idx[:])
                # gather local candidates' score rows from own patched bank.
                # Only the local top-5 can ever reach the global top-6 (the
                # 6th slot is always the self candidate from G), so 5 gathers
                # per row tile suffice.
                NG = 5
                scr = sp.tile([P, NG * C], F32, tag="scr", name="scr", bufs=2)
                for s in range(NG):
                    nc.gpsimd.indirect_dma_start(
                        out=scr[:, s * C:(s + 1) * C],
                        out_offset=None,
                        in_=sb_h.ap(),
                        in_offset=IndirectOffsetOnAxis(ap=bidx_u[:, s:s + 1],
                                                       axis=0),
                    )
                # kl_c = sum_c s*(ln s - pa)
                pa_bc = p_t["a"][m][:, None, :].to_broadcast([P, NG, C])
                lns = sp.tile([P, NG * C], F32, tag="lns", name="lns", bufs=2)
                nc.scalar.activation(lns[:], scr[:], AF.Ln)
                dd = sp.tile([P, NG * C], F32, tag="dd", name="dd", bufs=2)
                nc.vector.tensor_sub(
                    dd[:].rearrange("p (k c) -> p k c", k=NG),
                    lns[:].rearrange("p (k c) -> p k c", k=NG), pa_bc,
                )
                nc.vector.tensor_mul(dd[:], scr[:], dd[:])
                kc = sp.tile([P, 8], F32, tag="kc", name="kc", bufs=2)
                nc.vector.memset(kc[:, NG:8], 0.0)
                for s in range(NG):
                    nc.vector.reduce_sum(kc[:, s:s + 1], dd[:, s * C:(s + 1) * C],
                                         axis=AX.X)
                klc.append(kc)
                # enc2 = (fp16 val | global slot id)
                ee = sp.tile([P, 8], F32, tag="e2", name="e2", bufs=2)
                nc.gpsimd.partition_broadcast(ee[:].bitcast(U32),
                                              slotlo_sb[:, 0:8])
                nc.vector.tensor_copy(hi16(ee[:])[:, 0:5], hi16(lv8[:])[:, 0:5])
                e2.append(ee)
                dump(f"lv8_{m}", lv8[:])
                dump(f"idx_{m}", idx[:])
                dump(f"kc_{m}", kc[:])


            # ---------- the heavy stream ----------
            canv = [cp.tile([P, NWIN * 8], F32, tag=f"canv{m}", name=f"canv{m}")
                    for m in range(NM)]
            for w in range(NWIN):
                if w < 2:
                    fbw = fbw_all[w]
                else:
                    fbw = [fp.tile([P, 2, WWIN], FP8, tag=f"fbw{kp}",
                                   name=f"fbw{kp}_{w}") for kp in range(2)]
                    for kp in range(2):
                        nc.sync.dma_start(
                            fbw[kp][:],
                            fb8_h.ap()[kp * 256:(kp + 1) * 256,
                                       w * WWIN:(w + 1) * WWIN].rearrange(
                                "(s p) j -> p s j", s=2),
                        )
                for m in range(NM):
                    msl = slice(m * P, (m + 1) * P)
                    eb = ebuf[m][w % 2]
                    for tp in range(3):          # tile pairs (0,1)(2,3)(4,5)
                        t0 = 2 * tp
                        ps = ppA.tile([P, 2 * TN], F32, tag="ps_big",
                                      name="ps_big")
                        for ti in range(2):
                            t = t0 + ti
                            for kp in range(2):
                                nc.tensor.matmul(
                                    ps[:, ti * TN:(ti + 1) * TN],
                                    lhsT=tf8[kp][:, :, msl],
                                    rhs=fbw[kp][:, :, t * TN:(t + 1) * TN],
                                    start=(kp == 0), stop=(kp == 1),
                                    perf_mode=DR,
                                )
                        nc.scalar.copy(
                            hi16(eb[:])[:, t0 * TN:(t0 + 2) * TN].bitcast(F16),
                            ps[:],
                        )
                    # tile 6 on DVE (fp16 round like ACT)
                    ps1 = ppS.tile([P, TN], F32, tag="ps_one", name="ps_one")
                    for kp in range(2):
                        nc.tensor.matmul(
                            ps1[:],
                            lhsT=tf8[kp][:, :, msl],
                            rhs=fbw[kp][:, :, 6 * TN:7 * TN],
                            start=(kp == 0), stop=(kp == 1),
                            perf_mode=DR,
                        )
                    dst6 = hi16(eb[:])[:, 6 * TN:7 * TN].bitcast(F16)
                    if (w + m) % 2 == 0:
                        nc.vector.tensor_copy(dst6, ps1[:])
                    else:
                        nc.scalar.copy(dst6, ps1[:])
                    nc.vector.max(out=canv[m][:, w * 8:(w + 1) * 8], in_=eb[:])
                    if w == NWIN - 1:
                        local_merge(m)

            # ---------- ONE AllGather of (enc2, kl) ----------
            cand = dr.tile([NM * P, 16], F32, tag="cand", name="cand")
            bounce_h = nc.dram_tensor("bounce_sh", [NCORES * NM * P, 16], F32,
                                      kind="Internal", addr_space="Shared")
            bounce = bounce_h.ap()
            for m in range(NM):
                msl = slice(m * P, (m + 1) * P)
                nc.sync.dma_start(cand[msl, 0:8], e2[m][:])
                nc.sync.dma_start(cand[msl, 8:16], klc[m][:])
            nc.gpsimd.collective_compute(
                "AllGather",
                ALU.bypass,
                replica_groups=[list(range(NCORES))],
                ins=[cand[:].opt()],
                outs=[bounce.opt()],
            )

            # ---------- global merge: top-6 by enc2, select kl ----------
            sg = bounce.rearrange("(c q p) (h e) -> q p c h e",
                                  c=NCORES, q=NM, p=P, h=2)
            for m in range(NM):
                allE = sp.tile([P, TOT], F32, tag="allE", name="allE", bufs=2)
                allKL = sp.tile([P, TOT], F32, tag="allKL", name="allKL", bufs=2)
                nc.sync.dma_start(
                    allE[:, 0:64].rearrange("p (c e) -> p c e", c=NCORES),
                    sg[m, :, :, 0, :],
                )
                nc.sync.dma_start(
                    allKL[:, 0:64].rearrange("p (c e) -> p c e", c=NCORES),
                    sg[m, :, :, 1, :],
                )
                nc.gpsimd.partition_broadcast(
                    allE[:, 64:72].bitcast(U32), slotlo_sb[:, 8:16]
                )
                g16 = sp.tile([P, 8], F16, tag="g16", name="g16", bufs=2)
                nc.vector.tensor_copy(g16[:], Gtop[m][:])
                nc.vector.tensor_copy(hi16(allE[:, 64:72]), g16[:].bitcast(U16))
                nc.vector.tensor_copy(allKL[:, 64:72], klG[m][:])

                winE = sp.tile([P, 8], F32, tag="winE", name="winE", bufs=2)
                nc.vector.max(out=winE[:], in_=allE[:])
                winKL = sp.tile([P, K], F32, tag="winKL", name="winKL", bufs=2)
                for s in range(1, K + 1):
                    tmp = sp.tile([P, TOT], F32, tag="tmpm", name="tmpm", bufs=3)
                    red = sp.tile([P, 1], F32, tag="redm", name="redm", bufs=3)
                    nc.vector.scalar_tensor_tensor(
                        tmp[:], in0=allE[:], scalar=winE[:, s:s + 1],
                        in1=allKL[:], op0=ALU.is_equal, op1=ALU.mult,
                        accum_out=red[:],
                    )
                    nc.vector.tensor_copy(winKL[:, s - 1:s], red[:])
                klrow = sp.tile([P, 1], F32, tag="klrow", name="klrow", bufs=2)
                nc.vector.reduce_sum(klrow[:], winKL[:], axis=AX.X)
                nc.vector.tensor_copy(stat[m][:, 0:1], klrow[:])
                dump(f"allE{m}", allE[:])
                dump(f"allKL{m}", allKL[:])
                dump(f"winKL{m}", winKL[:])

            # ---------- final scalar reductions ----------
            ps_st = ppB.tile([1, 3], F32, tag="ps_small", name="ps_st")
            for m in range(NM):
                nc.tensor.matmul(
                    ps_st[:], lhsT=ones_m1[:], rhs=stat[m][:],
                    start=(m == 0), stop=(m == NM - 1),
                )
            stats = sp.tile([1, 3], F32, tag="stats", name="stats")
            nc.scalar.copy(stats[:], ps_st[:])

            t1 = sp.tile([1, 1], F32, tag="t1", name="t1")
            nc.vector.scalar_tensor_tensor(
                t1[:], in0=stats[:, 1:2], scalar=(-ENT_WT / B), in1=imsum[:],
                op0=ALU.mult, op1=ALU.add,
            )
            t2 = sp.tile([1, 1], F32, tag="t2", name="t2")
            nc.vector.scalar_tensor_tensor(
                t2[:], in0=stats[:, 0:1], scalar=(AAD_WT / B), in1=t1[:],
                op0=ALU.mult, op1=ALU.add,
            )
            t3 = sp.tile([1, 1], F32, tag="t3", name="t3")
            nc.vector.scalar_tensor_tensor(
                t3[:], in0=stats[:, 2:3], scalar=(TGT_WT / B), in1=t2[:],
                op0=ALU.mult, op1=ALU.add,
            )
            nc.sync.dma_start(loss_h.ap()[:, :], t3[:])

    nc.compile()
    return nc


def make_in_maps(trg_feat, W, b, W_aad, b_aad, fea_bank, score_bank, trg_idx):
    trg_feat = np.ascontiguousarray(np.asarray(trg_feat, dtype=np.float32))
    fea_bank = np.asarray(fea_bank, dtype=np.float32)
    score_bank = np.ascontiguousarray(np.asarray(score_bank, dtype=np.float32))
    trg_idx = np.asarray(trg_idx).astype(np.int64)

    gmask = np.zeros((B,), dtype=np.float32)
    seen = set()
    for j in range(B - 1, -1, -1):
        if int(trg_idx[j]) in seen:
            gmask[j] = BIGNEG
        else:
            seen.add(int(trg_idx[j]))

    tfT = np.ascontiguousarray(trg_feat.T)
    tf8 = np.ascontiguousarray(tfT.astype(ml_dtypes.float8_e4m3))
    fbT8 = np.ascontiguousarray(fea_bank.T.astype(ml_dtypes.float8_e4m3))

    common = {
        "gmask": gmask.reshape(1, B),
        "tfT": tfT,
        "tf8": tf8,
        "Wm": np.ascontiguousarray(np.asarray(W, dtype=np.float32)),
        "bm": np.asarray(b, dtype=np.float32).reshape(1, C),
        "Wa": np.ascontiguousarray(np.asarray(W_aad, dtype=np.float32)),
        "ba": np.asarray(b_aad, dtype=np.float32).reshape(1, C),
        "sbank": score_bank,
        "ramp": np.arange(WWIN, dtype=np.uint32).reshape(1, WWIN),
        "winoff": np.repeat(np.arange(NWIN, dtype=np.float32) * WWIN,
                            8).reshape(1, NWIN * 8),
        "eye": np.eye(P, dtype=np.float32),
        "tidxu": trg_idx.astype(np.uint32).reshape(B, 1),
    }
    in_maps = []
    for c in range(NCORES):
        sl = slice(c * NLOC, (c + 1) * NLOC)
        fb8 = np.zeros((D, NLOCP), dtype=ml_dtypes.float8_e4m3)
        fb8[:, :NLOC] = fbT8[:, sl]
        slotlo = np.concatenate([
            np.arange(c * 8, c * 8 + 8, dtype=np.uint32),
            np.arange(64, 72, dtype=np.uint32),
        ]).reshape(1, 16)
        in_maps.append(dict(
            common,
            fb8=fb8,
            slotlo=slotlo,
            core_base=np.full((P, 1), float(c * NLOC), dtype=np.float32),
        ))
    return in_maps


_cached_nc = None
last_results = None


def kernel(trg_feat, W, b, W_aad, b_aad, fea_bank, score_bank, trg_idx):
    global _cached_nc, last_results
    if _cached_nc is None:
        _cached_nc = build_program()
    in_maps = make_in_maps(
        trg_feat, W, b, W_aad, b_aad, fea_bank, score_bank, trg_idx
    )
    last_results = run_bass_kernel_spmd(
        _cached_nc, in_maps, core_ids=list(range(NCORES))
    )
    loss = np.asarray(last_results.results[0]["loss"], dtype=np.float32)
    return loss.reshape(())


# revision 16
# speedup vs baseline: 1.1265x; 1.1265x over previous
# Distributed kNN-retrieval loss kernel for Trainium2 (8 NeuronCores).
#
# Reference: two linear heads + softmax, feature bank updated at trg_idx
# (no-grad), cosine kNN against the bank, KL pseudo-label + entropy/IM +
# label-smoothed CE. Output: scalar loss.
#
# v3 strategy:
#  * Bank shipped fp8(e4m3) [D, 25088]/core (25000 rows + 88 zero-pad cols);
#    stream matmuls are DoubleRow fp8 (contraction 256/instr): half the PE
#    instructions and HBM bytes of bf16.
#  * Per-row positive scaling never changes a row's top-k order, so the
#    stream uses UNNORMALIZED trg_feat.
#  * Index-in-value encoding kills FIND_INDEX8: ACT (and DVE for the odd
#    tile) copy each PSUM dist tile as fp16 into the HIGH u16 halves of an
#    f32 canvas whose LOW halves hold the window-local column id. For the
#    positive values that matter, f32 order of the packed word ==
#    (fp16 dist, col) lexicographic: ONE MAX8 per 3584-col window returns
#    values AND indices, tie-free.
#  * KL-through-the-collective: each core decodes its local top-8, gathers
#    their score rows from its own (patched) score_bank and reduces them to
#    per-candidate KL contributions WHILE the AllGather rendezvous waits;
#    the single 16KB AllGather carries (enc2, kl) pairs where enc2 =
#    (fp16 val | unique global slot id). Post-collective work is just a
#    MAX8 + 5 exact-match selects - no index exchange, no post-merge
#    gathers.
#  * Bank update handled via G = fn@fn.T (exact, f32): all updated columns
#    compete as candidates with slots 64..71; their KL comes from
#    KLG[b,j] = H_a[j] - (Pa Pa^T)[b,j] selected by the same match trick
#    (zero gathers). Self column = global max, dropped: exactly
#    reference's top_k(K+1)[:,1:]. Stale stream columns stay (~2e-4).
#  * Entropy/IM/CE computed in the stream shadow; host reads core 0's loss.

import ml_dtypes
import numpy as np

import concourse.bass as bass
import concourse.mybir as mybir
import concourse.tile as tile
from concourse import bacc
from concourse.bass import IndirectOffsetOnAxis
from concourse.bass_utils import run_bass_kernel_spmd

F32 = mybir.dt.float32
F16 = mybir.dt.float16
BF16 = mybir.dt.bfloat16
FP8 = mybir.dt.float8e4
U32 = mybir.dt.uint32
U16 = mybir.dt.uint16
AF = mybir.ActivationFunctionType
ALU = mybir.AluOpType
AX = mybir.AxisListType
DR = mybir.MatmulPerfMode.DoubleRow

B = 256
D = 512
C = 10
N = 200000
K = 5
EPS_LS = 0.1
ENT_WT, IM_WT, AAD_WT, TGT_WT = 1.0, 1.0, 1.0, 0.1

P = 128
NM = B // P
NCORES = 8
NLOC = N // NCORES        # 25000
TN = 512
NTW = 7
WWIN = TN * NTW           # 3584
NWIN = 7
NLOCP = NWIN * WWIN       # 25088
TOT = NCORES * 8 + 8      # 72

BIGNEG = -1.0e30


def hi16(ap):
    """[P, n] f32 AP -> [P, n] u16 AP of the high halves."""
    return ap.bitcast(U16).rearrange("p (j two) -> p j two", two=2)[:, :, 1]


def lo16(ap):
    return ap.bitcast(U16).rearrange("p (j two) -> p j two", two=2)[:, :, 0]


def build_program(debug=False):
    nc = bacc.Bacc(
        "TRN2", target_bir_lowering=False, debug=False, num_devices=NCORES
    )

    fb8_h = nc.dram_tensor("fb8", [D, NLOCP], FP8, kind="ExternalInput")
    tf8_h = nc.dram_tensor("tf8", [D, B], FP8, kind="ExternalInput")
    tfT_h = nc.dram_tensor("tfT", [D, B], F32, kind="ExternalInput")
    cb_h = nc.dram_tensor("core_base", [P, 1], F32, kind="ExternalInput")
    gmask_h = nc.dram_tensor("gmask", [1, B], F32, kind="ExternalInput")
    Wm_h = nc.dram_tensor("Wm", [D, C], F32, kind="ExternalInput")
    bm_h = nc.dram_tensor("bm", [1, C], F32, kind="ExternalInput")
    Wa_h = nc.dram_tensor("Wa", [D, C], F32, kind="ExternalInput")
    ba_h = nc.dram_tensor("ba", [1, C], F32, kind="ExternalInput")
    sb_h = nc.dram_tensor("sbank", [N, C], F32, kind="ExternalInput")
    ramp_h = nc.dram_tensor("ramp", [1, WWIN], U32, kind="ExternalInput")
    slotlo_h = nc.dram_tensor("slotlo", [1, 16], U32, kind="ExternalInput")
    winoff_h = nc.dram_tensor("winoff", [1, NWIN * 8], F32, kind="ExternalInput")
    eye_h = nc.dram_tensor("eye", [P, P], F32, kind="ExternalInput")
    tidxu_h = nc.dram_tensor("tidxu", [B, 1], U32, kind="ExternalInput")
    loss_h = nc.dram_tensor("loss", [1, 1], F32, kind="ExternalOutput")

    def dump(name, ap):
        if not debug:
            return
        t = nc.dram_tensor(f"dbg_{name}", list(ap.shape), ap.dtype,
                           kind="ExternalOutput")
        nc.sync.dma_start(t.ap()[tuple(slice(0, d) for d in ap.shape)], ap)

    with tile.TileContext(nc) as tc:
        with (
            tc.tile_pool(name="const", bufs=1) as cp,
            tc.tile_pool(name="fbw", bufs=3) as fp,
            tc.tile_pool(name="scratch", bufs=2) as sp,
            tc.tile_pool(name="psA", bufs=2, space="PSUM") as ppA,
            tc.tile_pool(name="psS", bufs=2, space="PSUM") as ppS,
            tc.tile_pool(name="psB", bufs=1, space="PSUM") as ppB,
            tc.tile_pool(name="dram", bufs=1, space="DRAM") as dr,
        ):
            # ---------- gating transfers, fewest possible descriptors ----------
            # (Sync descgen is ~600ns per dma_start and fully serial, so the
            # stream-gating loads are consolidated and emitted first.)
            tf8_t = cp.tile([P, 4, B], FP8, tag="tf8", name="tf8")
            nc.sync.dma_start(
                tf8_t[:], tf8_h.ap().rearrange("(s p) b -> p s b", s=4),
            )
            tf8 = [tf8_t[:, 2 * kp:2 * kp + 2, :] for kp in range(2)]
            ramp_sb = cp.tile([1, WWIN], U32, tag="ramp", name="ramp")
            nc.sync.dma_start(ramp_sb[:], ramp_h.ap()[:, :])
            tfT_t = cp.tile([P, 4, B], F32, tag="tfT", name="tfT")
            nc.sync.dma_start(
                tfT_t[:], tfT_h.ap().rearrange("(s p) b -> p s b", s=4),
            )
            tfT = [tfT_t[:, k, :] for k in range(4)]
            Wsb = {}
            bsb = {}
            for name, Wh, bh in (("m", Wm_h, bm_h), ("a", Wa_h, ba_h)):
                Wt = cp.tile([P, 4, C], F32, tag=f"W{name}", name=f"W{name}")
                nc.sync.dma_start(
                    Wt[:], Wh.ap().rearrange("(s p) c -> p s c", s=4),
                )
                Wsb[name] = [Wt[:, k, :] for k in range(4)]
                bsb[name] = cp.tile([1, C], F32, tag=f"b{name}", name=f"b{name}")
                nc.sync.dma_start(bsb[name][:], bh.ap()[:, :])

            # window 0/1 bank data
            fbw_all = []
            HW2 = WWIN // 2
            for w in range(2):
                fbw = [fp.tile([P, 2, WWIN], FP8, tag=f"fbw{kp}",
                               name=f"fbw{kp}_{w}") for kp in range(2)]
                for h in range(2):      # halves: first tiles arrive sooner
                    for kp in range(2):
                        nc.sync.dma_start(
                            fbw[kp][:, :, h * HW2:(h + 1) * HW2],
                            fb8_h.ap()[kp * 256:(kp + 1) * 256,
                                       w * WWIN + h * HW2:
                                       w * WWIN + (h + 1) * HW2].rearrange(
                                "(s p) j -> p s j", s=2),
                        )
                fbw_all.append(fbw)

            # canvases' low halves (GpSimd, overlaps the DMAs above)
            ebuf = [[cp.tile([P, WWIN], F32, tag=f"ebuf{m}{bi}",
                             name=f"ebuf{m}{bi}") for bi in range(2)]
                    for m in range(NM)]
            for bi in range(2):          # buffer 0 first: window 0 needs it
                for m in range(NM):
                    nc.gpsimd.partition_broadcast(
                        ebuf[m][bi][:].bitcast(U32), ramp_sb[:]
                    )

            # late-use constants (after the stream-gating DMAs)
            gmask_sb = cp.tile([1, B], F32, tag="gmask", name="gmask")
            nc.sync.dma_start(gmask_sb[:], gmask_h.ap()[:, :])
            cb_sb = cp.tile([P, 1], F32, tag="cb", name="cb")
            nc.sync.dma_start(cb_sb[:], cb_h.ap()[:, :])
            tidxu_sb = [cp.tile([P, 1], U32, tag=f"tidxu{m}", name=f"tidxu{m}")
                        for m in range(NM)]
            for m in range(NM):
                nc.sync.dma_start(tidxu_sb[m][:], tidxu_h.ap()[m * P:(m + 1) * P, :])
            slotlo_sb = cp.tile([1, 16], U32, tag="slotlo", name="slotlo")
            nc.sync.dma_start(slotlo_sb[:], slotlo_h.ap()[:, :])
            winoff_sb = cp.tile([1, NWIN * 8], F32, tag="winoff", name="winoff")
            nc.sync.dma_start(winoff_sb[:], winoff_h.ap()[:, :])
            winoff_bc = cp.tile([P, NWIN * 8], F32, tag="winoff_bc",
                                name="winoff_bc")
            nc.gpsimd.partition_broadcast(winoff_bc[:], winoff_sb[:])
            eye_sb = cp.tile([P, P], F32, tag="eye", name="eye")
            nc.sync.dma_start(eye_sb[:], eye_h.ap()[:, :])

            ones_k1 = cp.tile([1, P], F32, tag="ones_k1", name="ones_k1")
            nc.vector.memset(ones_k1[:], 1.0)
            ones_m1 = cp.tile([P, 1], F32, tag="ones_m1", name="ones_m1")
            nc.vector.memset(ones_m1[:], 1.0)
            eps_b = cp.tile([P, 1], F32, tag="eps_b", name="eps_b")
            nc.vector.memset(eps_b[:], 1e-5)

            # ---------- classifier heads + softmax ----------
            p_t = {"m": [], "a": []}
            pmax_t = {"m": [], "a": []}
            logp_t = []
            for m in range(NM):
                msl = slice(m * P, (m + 1) * P)
                for name in ("m", "a"):
                    ps = ppB.tile([P, C], F32, tag="ps_small", name="ps_small")
                    for k in range(4):
                        nc.tensor.matmul(
                            ps[:], lhsT=tfT[k][:, msl], rhs=Wsb[name][k],
                            start=(k == 0), stop=False,
                        )
                    nc.tensor.matmul(
                        ps[:], lhsT=ones_k1[:], rhs=bsb[name][:],
                        start=False, stop=True,
                    )
                    lg = cp.tile([P, C], F32, tag=f"lg{name}{m}", name=f"lg{name}{m}")
                    nc.scalar.copy(lg[:], ps[:])
                    mx = cp.tile([P, 1], F32, tag=f"mx{name}{m}", name=f"mx{name}{m}")
                    nc.vector.reduce_max(mx[:], lg[:], axis=AX.X)
                    negmx = sp.tile([P, 1], F32, tag="negmx", name="negmx")
                    nc.vector.tensor_scalar_mul(negmx[:], mx[:], -1.0)
                    exps = sp.tile([P, C], F32, tag="exps", name="exps")
                    sumexp = cp.tile([P, 1], F32, tag=f"se{name}{m}",
                                     name=f"se{name}{m}")
                    nc.scalar.activation(
                        exps[:], lg[:], AF.Exp, bias=negmx[:], scale=1.0,
                        accum_out=sumexp[:],
                    )
                    rcp = sp.tile([P, 1], F32, tag="rcp", name="rcp")
                    nc.vector.reciprocal(rcp[:], sumexp[:])
                    pp = cp.tile([P, C], F32, tag=f"p{name}{m}", name=f"p{name}{m}")
                    nc.vector.tensor_scalar_mul(pp[:], exps[:], rcp[:])
                    p_t[name].append(pp)
                    pm = cp.tile([P, 1], F32, tag=f"pmax{name}{m}",
                                 name=f"pmax{name}{m}")
                    nc.vector.reduce_max(pm[:], pp[:], axis=AX.X)
                    pmax_t[name].append(pm)
                    if name == "m":
                        lnS = sp.tile([P, 1], F32, tag="lnS", name="lnS")
                        nc.scalar.activation(lnS[:], sumexp[:], AF.Ln)
                        logZ = sp.tile([P, 1], F32, tag="logZ", name="logZ")
                        nc.vector.tensor_add(logZ[:], lnS[:], mx[:])
                        lp = cp.tile([P, C], F32, tag=f"logp{m}", name=f"logp{m}")
                        nc.vector.tensor_scalar_sub(lp[:], lg[:], logZ[:])
                        logp_t.append(lp)

            # patch this core's score_bank: rows trg_idx <- p_aad
            for m in range(NM):
                nc.gpsimd.indirect_dma_start(
                    out=sb_h.ap(),
                    out_offset=IndirectOffsetOnAxis(ap=tidxu_sb[m][:], axis=0),
                    in_=p_t["a"][m][:],
                    in_offset=None,
                )

            # ---------- G = trg @ trg.T scaled to raw-dist column scale ----------
            ps2 = ppB.tile([1, B], F32, tag="ps_small", name="ps_s2")
            for k in range(4):
                sq = sp.tile([P, B], F32, tag="sq", name="sq")
                nc.scalar.square(sq[:], tfT[k])
                nc.tensor.matmul(
                    ps2[:], lhsT=ones_m1[:], rhs=sq[:],
                    start=(k == 0), stop=(k == 3),
                )
            srow = cp.tile([1, B], F32, tag="srow", name="srow")
            nc.scalar.sqrt(srow[:], ps2[:])
            invs = cp.tile([1, B], F32, tag="invs", name="invs")
            nc.vector.reciprocal(invs[:], srow[:])
            invs_bc = cp.tile([P, B], F32, tag="invs_bc", name="invs_bc")
            nc.gpsimd.partition_broadcast(invs_bc[:], invs[:])
            gm_bc = cp.tile([P, B], F32, tag="gm_bc", name="gm_bc")
            nc.gpsimd.partition_broadcast(gm_bc[:], gmask_sb[:])

            # Pa^T [C, B] via PE transpose (for Pa@Pa^T and H_a)
            paT = cp.tile([C, B], F32, tag="paT", name="paT")
            for m in range(NM):
                pst = ppB.tile([C, P], F32, tag="ps_small", name="ps_tr")
                nc.tensor.transpose(pst[:], p_t["a"][m][:], eye_sb[:])
                nc.scalar.copy(paT[:, m * P:(m + 1) * P], pst[:])
            # H_a[j] = sum_c pa[j,c] ln pa[j,c]  as a [1, B] row
            lnpaT = sp.tile([C, B], F32, tag="lnpaT", name="lnpaT")
            nc.scalar.activation(lnpaT[:], paT[:], AF.Ln)
            pelnT = sp.tile([C, B], F32, tag="pelnT", name="pelnT")
            nc.vector.tensor_mul(pelnT[:], paT[:], lnpaT[:])
            psH = ppB.tile([1, B], F32, tag="ps_small", name="ps_H")
            nc.tensor.matmul(psH[:], lhsT=ones_m1[0:C, :], rhs=pelnT[:],
                             start=True, stop=True)
            Ha = cp.tile([1, B], F32, tag="Ha", name="Ha")
            nc.scalar.copy(Ha[:], psH[:])
            Ha_bc = cp.tile([P, B], F32, tag="Ha_bc", name="Ha_bc")
            nc.gpsimd.partition_broadcast(Ha_bc[:], Ha[:])

            Gtop = []
            klG = []
            for m in range(NM):
                msl = slice(m * P, (m + 1) * P)
                psG = ppB.tile([P, B], F32, tag="ps_small", name="ps_G")
                for k in range(4):
                    nc.tensor.matmul(
                        psG[:], lhsT=tfT[k][:, msl], rhs=tfT[k],
                        start=(k == 0), stop=(k == 3),
                    )
                Gp = sp.tile([P, B], F32, tag="Gp", name="Gp")
                nc.vector.tensor_mul(Gp[:], psG[:], invs_bc[:])
                nc.vector.tensor_add(Gp[:], Gp[:], gm_bc[:])
                gt = cp.tile([P, 8], F32, tag=f"Gtop{m}", name=f"Gtop{m}")
                nc.vector.max(out=gt[:], in_=Gp[:])
                # KLG[b, j] = H_a[j] - (Pa Pa^T)[b, j]
                psPP = ppB.tile([P, B], F32, tag="ps_small", name="ps_PP")
                nc.tensor.matmul(psPP[:], lhsT=paT[:, msl], rhs=paT[:],
                                 start=True, stop=True)
                KLG = sp.tile([P, B], F32, tag="KLG", name="KLG")
                nc.vector.tensor_sub(KLG[:], Ha_bc[:], psPP[:])
                kg = cp.tile([P, 8], F32, tag=f"klG{m}", name=f"klG{m}")
                for gs in range(8):
                    gtmp = sp.tile([P, B], F32, tag="gtmp", name="gtmp")
                    gred = sp.tile([P, 1], F32, tag="gred", name="gred")
                    nc.vector.scalar_tensor_tensor(
                        gtmp[:], in0=Gp[:], scalar=gt[:, gs:gs + 1],
                        in1=KLG[:], op0=ALU.is_equal, op1=ALU.mult,
                        accum_out=gred[:],
                    )
                    nc.vector.tensor_copy(kg[:, gs:gs + 1], gred[:])
                Gtop.append(gt)
                klG.append(kg)
                dump(f"gtop{m}", gt[:])
                dump(f"klG{m}", kg[:])

            # ---------- entropy / IM / CE (stream shadow) ----------
            stat = [cp.tile([P, 3], F32, tag=f"stat{m}", name=f"stat{m}")
                    for m in range(NM)]
            for m in range(NM):
                lp5 = sp.tile([P, C], F32, tag="lp5", name="lp5")
                nc.scalar.activation(lp5[:], p_t["m"][m][:], AF.Ln, bias=eps_b[:])
                pe = sp.tile([P, C], F32, tag="pe", name="pe")
                nc.vector.tensor_mul(pe[:], p_t["m"][m][:], lp5[:])
                entneg = sp.tile([P, 1], F32, tag="entneg", name="entneg")
                nc.vector.reduce_sum(entneg[:], pe[:], axis=AX.X)
                nc.vector.tensor_copy(stat[m][:, 1:2], entneg[:])

                pickm = sp.tile([P, 1], U32, tag="pickm", name="pickm")
                nc.vector.tensor_tensor(
                    pickm[:], pmax_t["m"][m][:], pmax_t["a"][m][:], op=ALU.is_gt
                )
                chosen = sp.tile([P, C], F32, tag="chosen", name="chosen")
                nc.vector.select(
                    chosen[:], pickm[:].to_broadcast([P, C]),
                    p_t["m"][m][:], p_t["a"][m][:],
                )
                c8 = sp.tile([P, 8], F32, tag="c8", name="c8")
                nc.vector.max(out=c8[:], in_=chosen[:])
                ohlp = sp.tile([P, C], F32, tag="ohlp", name="ohlp")
                lpsel = sp.tile([P, 1], F32, tag="lpsel", name="lpsel")
                nc.vector.scalar_tensor_tensor(
                    ohlp[:], in0=chosen[:], scalar=c8[:, 0:1],
                    in1=logp_t[m][:], op0=ALU.is_equal, op1=ALU.mult,
                    accum_out=lpsel[:],
                )
                slogp = sp.tile([P, 1], F32, tag="slogp", name="slogp")
                nc.vector.reduce_sum(slogp[:], logp_t[m][:], axis=AX.X)
                sl001 = sp.tile([P, 1], F32, tag="sl001", name="sl001")
                nc.vector.tensor_scalar_mul(sl001[:], slogp[:], EPS_LS / C)
                cerow = sp.tile([P, 1], F32, tag="cerow", name="cerow")
                nc.vector.scalar_tensor_tensor(
                    cerow[:], in0=lpsel[:], scalar=(1.0 - EPS_LS), in1=sl001[:],
                    op0=ALU.mult, op1=ALU.add,
                )
                nc.vector.tensor_scalar_mul(cerow[:], cerow[:], -1.0)
                nc.vector.tensor_copy(stat[m][:, 2:3], cerow[:])

            ps_mp = ppB.tile([1, C], F32, tag="ps_small", name="ps_mp")
            for m in range(NM):
                nc.tensor.matmul(
                    ps_mp[:], lhsT=ones_m1[:], rhs=p_t["m"][m][:],
                    start=(m == 0), stop=(m == NM - 1),
                )
            mp = cp.tile([1, C], F32, tag="mp", name="mp")
            nc.scalar.mul(mp[:], ps_mp[:], 1.0 / B)
            mplog = sp.tile([1, C], F32, tag="mplog", name="mplog")
            nc.scalar.activation(mplog[:], mp[:], AF.Ln, bias=eps_b[0:1, :])
            mpe = sp.tile([1, C], F32, tag="mpe", name="mpe")
            nc.vector.tensor_mul(mpe[:], mp[:], mplog[:])
            imsum = cp.tile([1, 1], F32, tag="imsum", name="imsum")
            nc.vector.reduce_sum(imsum[:], mpe[:], axis=AX.X)

            # ---------- local merge + per-candidate KL (pre-collective) ----------
            e2 = []
            klc = []
            cand_m = [dr.tile([P, 16], F32, tag=f"cand{m}", name=f"cand{m}")
                      for m in range(NM)]
            bounce_m = [
                nc.dram_tensor(f"bounce_sh{m}", [NCORES * P, 16], F32,
                               kind="Internal", addr_space="Shared").ap()
                for m in range(NM)
            ]
            def local_merge(m):
                lv8 = sp.tile([P, 8], F32, tag="lv8", name="lv8", bufs=2)
                nc.vector.max(out=lv8[:], in_=canv[m][:])
                woff = sp.tile([P, 8], F32, tag="woff", name="woff", bufs=2)
                for s in range(5):
                    tmp = sp.tile([P, NWIN * 8], F32, tag="tmpw", name="tmpw",
                                  bufs=3)
                    red = sp.tile([P, 1], F32, tag="redw", name="redw", bufs=3)
                    nc.vector.scalar_tensor_tensor(
                        tmp[:], in0=canv[m][:], scalar=lv8[:, s:s + 1],
                        in1=winoff_bc[:], op0=ALU.is_equal, op1=ALU.mult,
                        accum_out=red[:],
                    )
                    nc.vector.tensor_copy(woff[:, s:s + 1], red[:])
                jlocf = sp.tile([P, 5], F32, tag="jlocf", name="jlocf", bufs=2)
                nc.vector.tensor_copy(jlocf[:], lo16(lv8[:])[:, 0:5])
                idx = sp.tile([P, 5], F32, tag="idx", name="idx", bufs=2)
                nc.vector.tensor_add(idx[:], jlocf[:], woff[:, 0:5])
                nc.vector.tensor_scalar_add(idx[:], idx[:], cb_sb[:])
                nc.vector.tensor_scalar_min(idx[:], idx[:], float(N - 1))
                bidx_u = sp.tile([P, 5], U32, tag="bidxu", name="bidxu", bufs=2)
                nc.vector.tensor_copy(bidx_u[:], idx[:])
                # gather local candidates' score rows from own patched bank.
                # Only the local top-5 can ever reach the global top-6 (the
                # 6th slot is always the self candidate from G), so 5 gathers
                # per row tile suffice.
                NG = 5
                scr = sp.tile([P, NG * C], F32, tag="scr", name="scr", bufs=2)
                for s in range(NG):
                    nc.gpsimd.indirect_dma_start(
                        out=scr[:, s * C:(s + 1) * C],
                        out_offset=None,
                        in_=sb_h.ap(),
                        in_offset=IndirectOffsetOnAxis(ap=bidx_u[:, s:s + 1],
                                                       axis=0),
                    )
                # kl_c = sum_c s*(ln s - pa)
                pa_bc = p_t["a"][m][:, None, :].to_broadcast([P, NG, C])
                lns = sp.tile([P, NG * C], F32, tag="lns", name="lns", bufs=2)
                nc.scalar.activation(lns[:], scr[:], AF.Ln)
                dd = sp.tile([P, NG * C], F32, tag="dd", name="dd", bufs=2)
                nc.vector.tensor_sub(
                    dd[:].rearrange("p (k c) -> p k c", k=NG),
                    lns[:].rearrange("p (k c) -> p k c", k=NG), pa_bc,
                )
                nc.vector.tensor_mul(dd[:], scr[:], dd[:])
                kc = sp.tile([P, 8], F32, tag="kc", name="kc", bufs=2)
                nc.vector.memset(kc[:, NG:8], 0.0)
                for s in range(NG):
                    nc.vector.reduce_sum(kc[:, s:s + 1], dd[:, s * C:(s + 1) * C],
                                         axis=AX.X)
                klc.append(kc)
                # enc2 = (fp16 val | global slot id)
                ee = sp.tile([P, 8], F32, tag="e2", name="e2", bufs=2)
                nc.gpsimd.partition_broadcast(ee[:].bitcast(U32),
                                              slotlo_sb[:, 0:8])
                nc.vector.tensor_copy(hi16(ee[:])[:, 0:5], hi16(lv8[:])[:, 0:5])
                e2.append(ee)
                # ship this row-tile's candidates immediately: m0's collective
                # absorbs the inter-core skew while m1 is still merging
                nc.sync.dma_start(cand_m[m][:, 0:8], ee[:])
                nc.sync.dma_start(cand_m[m][:, 8:16], kc[:])
                nc.gpsimd.collective_compute(
                    "AllGather",
                    ALU.bypass,
                    replica_groups=[list(range(NCORES))],
                    ins=[cand_m[m][:].opt()],
                    outs=[bounce_m[m].opt()],
                )
                dump(f"lv8_{m}", lv8[:])
                dump(f"idx_{m}", idx[:])
                dump(f"kc_{m}", kc[:])


            # ---------- the heavy stream ----------
            canv = [cp.tile([P, NWIN * 8], F32, tag=f"canv{m}", name=f"canv{m}")
                    for m in range(NM)]
            for w in range(NWIN):
                if w < 2:
                    fbw = fbw_all[w]
                else:
                    fbw = [fp.tile([P, 2, WWIN], FP8, tag=f"fbw{kp}",
                                   name=f"fbw{kp}_{w}") for kp in range(2)]
                    for kp in range(2):
                        nc.sync.dma_start(
                            fbw[kp][:],
                            fb8_h.ap()[kp * 256:(kp + 1) * 256,
                                       w * WWIN:(w + 1) * WWIN].rearrange(
                                "(s p) j -> p s j", s=2),
                        )
                for m in range(NM):
                    msl = slice(m * P, (m + 1) * P)
                    eb = ebuf[m][w % 2]
                    for tp in range(3):          # tile pairs (0,1)(2,3)(4,5)
                        t0 = 2 * tp
                        ps = ppA.tile([P, 2 * TN], F32, tag="ps_big",
                                      name="ps_big")
                        for ti in range(2):
                            t = t0 + ti
                            for kp in range(2):
                                nc.tensor.matmul(
                                    ps[:, ti * TN:(ti + 1) * TN],
                                    lhsT=tf8[kp][:, :, msl],
                                    rhs=fbw[kp][:, :, t * TN:(t + 1) * TN],
                                    start=(kp == 0), stop=(kp == 1),
                                    perf_mode=DR,
                                )
                        nc.scalar.copy(
                            hi16(eb[:])[:, t0 * TN:(t0 + 2) * TN].bitcast(F16),
                            ps[:],
                        )
                    # tile 6 on DVE (fp16 round like ACT)
                    ps1 = ppS.tile([P, TN], F32, tag="ps_one", name="ps_one")
                    for kp in range(2):
                        nc.tensor.matmul(
                            ps1[:],
                            lhsT=tf8[kp][:, :, msl],
                            rhs=fbw[kp][:, :, 6 * TN:7 * TN],
                            start=(kp == 0), stop=(kp == 1),
                            perf_mode=DR,
                        )
                    dst6 = hi16(eb[:])[:, 6 * TN:7 * TN].bitcast(F16)
                    if (w + m) % 2 == 0:
                        nc.vector.tensor_copy(dst6, ps1[:])
                    else:
                        nc.scalar.copy(dst6, ps1[:])
                    nc.vector.max(out=canv[m][:, w * 8:(w + 1) * 8], in_=eb[:])
                    if w == NWIN - 1:
                        local_merge(m)

            # ---------- global merge: top-6 by enc2, select kl ----------
            for m in range(NM):
                sg = bounce_m[m].rearrange("(c p) (h e) -> p c h e",
                                           c=NCORES, p=P, h=2)
                allE = sp.tile([P, TOT], F32, tag="allE", name="allE", bufs=2)
                allKL = sp.tile([P, TOT], F32, tag="allKL", name="allKL", bufs=2)
                nc.sync.dma_start(
                    allE[:, 0:64].rearrange("p (c e) -> p c e", c=NCORES),
                    sg[:, :, 0, :],
                )
                nc.sync.dma_start(
                    allKL[:, 0:64].rearrange("p (c e) -> p c e", c=NCORES),
                    sg[:, :, 1, :],
                )
                nc.gpsimd.partition_broadcast(
                    allE[:, 64:72].bitcast(U32), slotlo_sb[:, 8:16]
                )
                g16 = sp.tile([P, 8], F16, tag="g16", name="g16", bufs=2)
                nc.vector.tensor_copy(g16[:], Gtop[m][:])
                nc.vector.tensor_copy(hi16(allE[:, 64:72]), g16[:].bitcast(U16))
                nc.vector.tensor_copy(allKL[:, 64:72], klG[m][:])

                winE = sp.tile([P, 8], F32, tag="winE", name="winE", bufs=2)
                nc.vector.max(out=winE[:], in_=allE[:])
                winKL = sp.tile([P, K], F32, tag="winKL", name="winKL", bufs=2)
                for s in range(1, K + 1):
                    tmp = sp.tile([P, TOT], F32, tag="tmpm", name="tmpm", bufs=3)
                    red = sp.tile([P, 1], F32, tag="redm", name="redm", bufs=3)
                    nc.vector.scalar_tensor_tensor(
                        tmp[:], in0=allE[:], scalar=winE[:, s:s + 1],
                        in1=allKL[:], op0=ALU.is_equal, op1=ALU.mult,
                        accum_out=red[:],
                    )
                    nc.vector.tensor_copy(winKL[:, s - 1:s], red[:])
                klrow = sp.tile([P, 1], F32, tag="klrow", name="klrow", bufs=2)
                nc.vector.reduce_sum(klrow[:], winKL[:], axis=AX.X)
                nc.vector.tensor_copy(stat[m][:, 0:1], klrow[:])
                dump(f"allE{m}", allE[:])
                dump(f"allKL{m}", allKL[:])
                dump(f"winKL{m}", winKL[:])

            # ---------- final scalar reductions ----------
            ps_st = ppB.tile([1, 3], F32, tag="ps_small", name="ps_st")
            for m in range(NM):
                nc.tensor.matmul(
                    ps_st[:], lhsT=ones_m1[:], rhs=stat[m][:],
                    start=(m == 0), stop=(m == NM - 1),
                )
            stats = sp.tile([1, 3], F32, tag="stats", name="stats")
            nc.scalar.copy(stats[:], ps_st[:])

            t1 = sp.tile([1, 1], F32, tag="t1", name="t1")
            nc.vector.scalar_tensor_tensor(
                t1[:], in0=stats[:, 1:2], scalar=(-ENT_WT / B), in1=imsum[:],
                op0=ALU.mult, op1=ALU.add,
            )
            t2 = sp.tile([1, 1], F32, tag="t2", name="t2")
            nc.vector.scalar_tensor_tensor(
                t2[:], in0=stats[:, 0:1], scalar=(AAD_WT / B), in1=t1[:],
                op0=ALU.mult, op1=ALU.add,
            )
            t3 = sp.tile([1, 1], F32, tag="t3", name="t3")
            nc.vector.scalar_tensor_tensor(
                t3[:], in0=stats[:, 2:3], scalar=(TGT_WT / B), in1=t2[:],
                op0=ALU.mult, op1=ALU.add,
            )
            nc.sync.dma_start(loss_h.ap()[:, :], t3[:])

    nc.compile()
    return nc


def make_in_maps(trg_feat, W, b, W_aad, b_aad, fea_bank, score_bank, trg_idx):
    trg_feat = np.ascontiguousarray(np.asarray(trg_feat, dtype=np.float32))
    fea_bank = np.asarray(fea_bank, dtype=np.float32)
    score_bank = np.ascontiguousarray(np.asarray(score_bank, dtype=np.float32))
    trg_idx = np.asarray(trg_idx).astype(np.int64)

    gmask = np.zeros((B,), dtype=np.float32)
    seen = set()
    for j in range(B - 1, -1, -1):
        if int(trg_idx[j]) in seen:
            gmask[j] = BIGNEG
        else:
            seen.add(int(trg_idx[j]))

    tfT = np.ascontiguousarray(trg_feat.T)
    tf8 = np.ascontiguousarray(tfT.astype(ml_dtypes.float8_e4m3))
    fbT8 = np.ascontiguousarray(fea_bank.T.astype(ml_dtypes.float8_e4m3))

    common = {
        "gmask": gmask.reshape(1, B),
        "tfT": tfT,
        "tf8": tf8,
        "Wm": np.ascontiguousarray(np.asarray(W, dtype=np.float32)),
        "bm": np.asarray(b, dtype=np.float32).reshape(1, C),
        "Wa": np.ascontiguousarray(np.asarray(W_aad, dtype=np.float32)),
        "ba": np.asarray(b_aad, dtype=np.float32).reshape(1, C),
        "sbank": score_bank,
        "ramp": np.arange(WWIN, dtype=np.uint32).reshape(1, WWIN),
        "winoff": np.repeat(np.arange(NWIN, dtype=np.float32) * WWIN,
                            8).reshape(1, NWIN * 8),
        "eye": np.eye(P, dtype=np.float32),
        "tidxu": trg_idx.astype(np.uint32).reshape(B, 1),
    }
    in_maps = []
    for c in range(NCORES):
        sl = slice(c * NLOC, (c + 1) * NLOC)
        fb8 = np.zeros((D, NLOCP), dtype=ml_dtypes.float8_e4m3)
        fb8[:, :NLOC] = fbT8[:, sl]
        slotlo = np.concatenate([
            np.arange(c * 8, c * 8 + 8, dtype=np.uint32),
            np.arange(64, 72, dtype=np.uint32),
        ]).reshape(1, 16)
        in_maps.append(dict(
            common,
            fb8=fb8,
            slotlo=slotlo,
            core_base=np.full((P, 1), float(c * NLOC), dtype=np.float32),
        ))
    return in_maps


_cached_nc = None
last_results = None


def kernel(trg_feat, W, b, W_aad, b_aad, fea_bank, score_bank, trg_idx):
    global _cached_nc, last_results
    if _cached_nc is None:
        _cached_nc = build_program()
    in_maps = make_in_maps(
        trg_feat, W, b, W_aad, b_aad, fea_bank, score_bank, trg_idx
    )
    last_results = run_bass_kernel_spmd(
        _cached_nc, in_maps, core_ids=list(range(NCORES))
    )
    loss = np.asarray(last_results.results[0]["loss"], dtype=np.float32)
    return loss.reshape(())
